# revision 8
# baseline (speedup 1.0000x reference)
"""Trainium2 Bass kernel for AdaptiveProjection (dense MoE routing), fp8.

Computes: out[t,:] = sum_e softmax(x@gate_w.T + gate_b)[t,e] * (x[t] @ W_e.T)

Strategy:
- Data-parallel over tokens across 8 cores; weights replicated.
- Expert matmuls in fp8 e4m3 with DoubleRow perf mode (2x PE rate).
- Accuracy recovery (fp8 alone is ~3.2e-2 rel err, gate is 2e-2):
  * Tokens are routed (host-side permutation) so each core's tokens
    mostly share the same top-1 expert ("designated expert" = core//2).
  * One extra fp8 correction matmul per tile contracts the pair
    (dx8, x8) against (A, B), where dx8 = fp8 residual of x,
    A = sum_e m_e W_e (conditional-mean gates m given the designated
    top expert) and B = sum_e m_e (W_e - fp8(W_e)). This cancels the
    conditional-mean component of both x- and W-quantization errors.
  * Gate logits are computed from (x8 + dx8) @ (gw8 + dgw8) so gate
    precision stays bf16-class.
  Emulated end-to-end rel err: 1.2e-2.
"""

import numpy as np
import ml_dtypes

B, S, D, O, E = 4, 4096, 1024, 1024, 4
N = B * S
N_CORES = 8
T = N // N_CORES        # 2048 tokens per core
KC = D // 128           # 8 contraction chunks of 128
KP = KC // 2            # 4 DoubleRow k-pairs
NT = T // 128           # 16 token tiles per core
NH = O // 512           # 2 output halves
GBLK = 512              # gate-logit token block
GP = 64                 # padded gate-stationary columns (DoubleRow needs 32/64/128)

FP8 = ml_dtypes.float8_e4m3
BF16 = ml_dtypes.bfloat16
S_DX = 64.0             # dx8 = fp8(64*(x - x8))
S_W = 64.0              # W8 = fp8(64*W)
S_A = 8.0               # A8 = fp8(8*A)   -> corr j0 scale 64*8 = 512
S_B = 512.0             # B8 = fp8(512*B) -> corr j1 scale 512
S_GW = 8.0              # gw8 = fp8(8*gw)
S_DGW = 512.0           # dgw8 = fp8(512*(gw - gw8/8))

_CACHE = {}


def _build_graph():
    import concourse.mybir as mybir
    from concourse import bacc
    from concourse.bass import ts, ds
    from concourse.tile import TileContext

    f32 = mybir.dt.float32
    bf16 = mybir.dt.bfloat16
    fp8 = mybir.dt.float8e4
    DR = mybir.MatmulPerfMode.DoubleRow
    nc = bacc.Bacc(None, target_bir_lowering=False)

    xi_d = nc.declare_dram_parameter("xi", [KC, 128, 2, T], fp8, isOutput=False)
    wt_d = nc.declare_dram_parameter("wt", [NH, KP, 128, 2, E, 512], fp8, isOutput=False)
    wc_d = nc.declare_dram_parameter("wc", [NH, KC, 128, 2, 512], fp8, isOutput=False)
    gs_d = nc.declare_dram_parameter("gs", [KP, 128, 2, 2, GP], fp8, isOutput=False)
    gb_d = nc.declare_dram_parameter("gb", [E, 1], f32, isOutput=False)
    cst_d = nc.declare_dram_parameter("cst", [E, 1], f32, isOutput=False)
    id_d = nc.declare_dram_parameter("ident", [E, E], bf16, isOutput=False)
    out_d = nc.declare_dram_parameter("out", [T, O], bf16, isOutput=True)

    with TileContext(nc) as tc:
        with (
            tc.tile_pool(name="persist", bufs=1) as pp,
            tc.tile_pool(name="gate_sm", bufs=4) as gp,
            tc.tile_pool(name="acc", bufs=8) as ap,
        ):
            # --- persistent SBUF tensors ---
            xi_sb = pp.tile([128, KC, 2, T], fp8, tag="xi")
            w_sb = pp.tile([128, NH, KP, 2, E, 512], fp8, tag="w")
            wc_sb = pp.tile([128, NH, KC, 2, 512], fp8, tag="wc")
            gs_sb = pp.tile([128, KP, 2, 2, GP], fp8, tag="gs")
            gb_sb = pp.tile([E, 1], f32, tag="gb")
            cst_sb = pp.tile([E, 1], f32, tag="cst")
            id_sb = pp.tile([E, E], bf16, tag="ident")
            exp_sb = pp.tile([E, T], bf16, tag="exprow")
            gates_sb = pp.tile([128, NT * E], f32, tag="gates")

            # --- loads ---
            # Big streams ride the sync ring only; tiny tensors go on the
            # scalar ring early (ACT stays free later for the combine).
            scratch = pp.tile([128, 512], bf16, tag="scratch")
            nc.vector.memset(scratch[:, :], 0)
            nc.scalar.dma_start(out=gb_sb[:, :], in_=gb_d[:, :])
            nc.scalar.dma_start(out=cst_sb[:, :], in_=cst_d[:, :])
            nc.scalar.dma_start(out=id_sb[:, :], in_=id_d[:, :])
            # split big loads across two DMA queues (sync + gpsimd):
            # sync carries gs + even xi chunks + first-half weights, gpsimd
            # the rest, so the t=0 correction group unblocks ~2x sooner.
            for kp in range(KP):
                nc.sync.dma_start(out=gs_sb[:, kp, :, :, :], in_=gs_d[kp])
            for k in range(KC):
                ring = nc.sync if k % 2 == 0 else nc.gpsimd
                ring.dma_start(out=xi_sb[:, k, :, :], in_=xi_d[k])
            for k in range(KC):
                ring = nc.sync if k % 2 == 0 else nc.gpsimd
                ring.dma_start(out=wc_sb[:, 0, k, :, :], in_=wc_d[0, k])
            for kp in range(KP):
                ring = nc.sync if kp % 2 == 0 else nc.gpsimd
                ring.dma_start(out=w_sb[:, 0, kp, :, :, :], in_=wt_d[0, kp])
            for k in range(KC):
                ring = nc.sync if k % 2 == 0 else nc.gpsimd
                ring.dma_start(out=wc_sb[:, 1, k, :, :], in_=wc_d[1, k])
            for kp in range(KP):
                ring = nc.sync if kp % 2 == 0 else nc.gpsimd
                ring.dma_start(out=w_sb[:, 1, kp, :, :, :], in_=wt_d[1, kp])

            # --- gate prologue ---
            NB = T // GBLK
            with (
                tc.tile_pool(name="psum_w", bufs=1, space="PSUM") as pwp,
                tc.tile_pool(name="psum_g", bufs=2, space="PSUM") as pgp,
                tc.tile_pool(name="psum_t", bufs=1, space="PSUM") as ptp,
            ):
                warm_ps = pwp.tile([128, 512], f32, tag="warm")
                for _ in range(12):
                    nc.tensor.matmul(
                        warm_ps[:, :],
                        scratch[:, 0:128],
                        scratch[:, :],
                        start=True,
                        stop=True,
                        skip_group_check=True,
                    )
                # logits blocks: pa rows 0..3 = x8@gw8 (scale 8), rows 4..7 =
                # x8@dgw8 (scale 512); pb rows 0..3 = dx8@gw8 (scale 512).
                for wave in range(NB // 2):
                    blocks = (2 * wave, 2 * wave + 1)
                    pas = {b: pgp.tile([GP, GBLK], f32, tag="pa",
                                       name=f"pa{b}") for b in blocks}
                    pbs = {b: pgp.tile([GP, GBLK], f32, tag="pb",
                                       name=f"pb{b}") for b in blocks}
                    # pa = x8@gw8 (scale 8); pb = dx8@gw8 + x8@dgw8
                    # (both scale 512, one accumulation group)
                    for kp in range(KP):
                        for b in blocks:
                            nc.tensor.matmul(
                                pas[b][:, :],
                                gs_sb[:, kp, :, 0, :],
                                xi_sb[:, 2 * kp : 2 * kp + 2, 1, ts(b, GBLK)],
                                start=(kp == 0),
                                stop=(kp == KP - 1),
                                perf_mode=DR,
                            )
                        for b in blocks:
                            nc.tensor.matmul(
                                pbs[b][:, :],
                                gs_sb[:, kp, :, 0, :],
                                xi_sb[:, 2 * kp : 2 * kp + 2, 0, ts(b, GBLK)],
                                start=(kp == 0),
                                stop=False,
                                perf_mode=DR,
                            )
                    for kp in range(KP):
                        for b in blocks:
                            nc.tensor.matmul(
                                pbs[b][:, :],
                                gs_sb[:, kp, :, 1, :],
                                xi_sb[:, 2 * kp : 2 * kp + 2, 1, ts(b, GBLK)],
                                start=False,
                                stop=(kp == KP - 1),
                                perf_mode=DR,
                            )
                    # assemble logits and exp() per block
                    for b in blocks:
                        tb = gp.tile([E, GBLK], f32, tag="tb")
                        t1 = gp.tile([E, GBLK], f32, tag="t1")
                        nc.vector.tensor_copy(tb[:, :], pbs[b][0:E, :])
                        # t1 = (tb * 1/64) + pa[0:4]  (units of scale 8)
                        nc.vector.scalar_tensor_tensor(
                            t1[:, :],
                            tb[:, :],
                            cst_sb[:, 0:1],
                            pas[b][0:E, :],
                            op0=mybir.AluOpType.mult,
                            op1=mybir.AluOpType.add,
                        )
                        nc.scalar.activation(
                            exp_sb[:, ts(b, GBLK)],
                            t1[:, :],
                            mybir.ActivationFunctionType.Exp,
                            bias=gb_sb[:, 0:1],
                            scale=0.125,
                        )
                # transpose exp rows -> [128, E] per token tile
                expT = ptp.tile([128, NT * E], bf16, tag="expT")
                for t in range(NT):
                    nc.tensor.transpose(
                        expT[:, ts(t, E)],
                        exp_sb[:, ts(t, 128)],
                        id_sb[:, :],
                    )
                denom = gp.tile([128, NT], f32, tag="denom")
                recip = gp.tile([128, NT], f32, tag="recip")
                expT3 = expT[:, :].rearrange("p (t e) -> p t e", e=E)
                nc.vector.reduce_sum(
                    denom[:, :], expT3, axis=mybir.AxisListType.X
                )
                # recip = 1/(64*denom) so gates_sb holds g/64 (expert psums
                # carry the 64x weight scale)
                nc.scalar.activation(
                    denom[:, :],
                    denom[:, :],
                    mybir.ActivationFunctionType.Copy,
                    bias=0.0,
                    scale=64.0,
                )
                nc.vector.reciprocal(recip[:, :], denom[:, :])
                nc.vector.tensor_mul(
                    gates_sb[:, :].rearrange("p (t e) -> p t e", e=E),
                    expT3,
                    recip[:, :, None].broadcast_to([128, NT, E]),
                )

            # --- main loop: corr + expert matmuls, gated combine ---
            with (
                tc.tile_pool(name="psum_c", bufs=2, space="PSUM") as pcp,
                tc.tile_pool(name="psum_e", bufs=6, space="PSUM") as pep,
            ):
                for h in range(NH):
                    for t in range(NT):
                        pc = pcp.tile([128, 512], f32, tag="pc",
                                      name=f"pc{t}_{h}")
                        psums = [
                            pep.tile([128, 512], f32, tag="ep",
                                     name=f"ep{t}_{h}_{e}")
                            for e in range(E)
                        ]
                        # correction first so its bank drains early
                        for k in range(KC):
                            nc.tensor.matmul(
                                pc[:, :],
                                xi_sb[:, k, :, ts(t, 128)],
                                wc_sb[:, h, k, :, :],
                                start=(k == 0),
                                stop=(k == KC - 1),
                                perf_mode=DR,
                            )
                        for kp in range(KP):
                            lhs = xi_sb[:, 2 * kp : 2 * kp + 2, 1, ts(t, 128)]
                            for e in range(E):
                                nc.tensor.matmul(
                                    psums[e][:, :],
                                    lhs,
                                    w_sb[:, h, kp, :, e, :],
                                    start=(kp == 0),
                                    stop=(kp == KP - 1),
                                    perf_mode=DR,
                                )
                        # combine: out = sum_e g_e*p_e + pc/512
                        g0 = gates_sb[:, t * E + 0 : t * E + 1]
                        g1 = gates_sb[:, t * E + 1 : t * E + 2]
                        g2 = gates_sb[:, t * E + 2 : t * E + 3]
                        g3 = gates_sb[:, t * E + 3 : t * E + 4]
                        ca = ap.tile([128, 512], f32, tag="ca")
                        cb = ap.tile([128, 512], f32, tag="cb")
                        acc = ap.tile([128, 512], bf16, tag="acc")
                        nc.scalar.activation(
                            ca[:, :],
                            pc[:, :],
                            mybir.ActivationFunctionType.Copy,
                            bias=0.0,
                            scale=1.0 / 512.0,
                        )
                        nc.vector.scalar_tensor_tensor(
                            ca[:, :], psums[0][:, :], g0, ca[:, :],
                            op0=mybir.AluOpType.mult,
                            op1=mybir.AluOpType.add,
                        )
                        nc.scalar.activation(
                            cb[:, :],
                            psums[1][:, :],
                            mybir.ActivationFunctionType.Copy,
                            bias=0.0,
                            scale=g1,
                        )
                        nc.vector.scalar_tensor_tensor(
                            cb[:, :], psums[2][:, :], g2, cb[:, :],
                            op0=mybir.AluOpType.mult,
                            op1=mybir.AluOpType.add,
                        )
                        nc.vector.scalar_tensor_tensor(
                            ca[:, :], psums[3][:, :], g3, ca[:, :],
                            op0=mybir.AluOpType.mult,
                            op1=mybir.AluOpType.add,
                        )
                        nc.vector.tensor_add(acc[:, :], ca[:, :], cb[:, :])
                        nc.scalar.dma_start(
                            out=out_d[ts(t, 128), ds(512 * h, 512)],
                            in_=acc[:, :],
                        )
    nc.compile()
    return nc


def _prep_inputs(x, W_experts, gate_w, gate_b):
    x_flat = np.asarray(x, dtype=np.float32).reshape(N, D)
    Wf = np.asarray(W_experts, dtype=np.float32)        # [E, O, D]
    gwf = np.asarray(gate_w, dtype=np.float32)          # [E, D]
    gbf = np.asarray(gate_b, dtype=np.float32)          # [E]

    # host routing: top-1 expert per token (layout decision only)
    logits = x_flat @ gwf.T + gbf
    top = np.argmax(logits, -1)
    gh = np.exp(logits - logits.max(-1, keepdims=True))
    gh /= gh.sum(-1, keepdims=True)
    m_top = float(gh[np.arange(N), top].mean())
    m_off = (1.0 - m_top) / 3.0

    core_tokens = [None] * N_CORES
    spill = []
    for e in range(E):
        toks = np.where(top == e)[0]
        take = toks[: 2 * T]
        spill.append(toks[2 * T :])
        core_tokens[2 * e] = take[:T]
        core_tokens[2 * e + 1] = take[T : 2 * T]
    spill = np.concatenate(spill) if spill else np.empty(0, np.int64)
    for c in range(N_CORES):
        need = T - len(core_tokens[c])
        if need > 0:
            core_tokens[c] = np.concatenate([core_tokens[c], spill[:need]])
            spill = spill[need:]
    perm = np.concatenate(core_tokens)

    # weights (shared across cores)
    W8 = (Wf * S_W).astype(FP8)                         # [E, O, D]
    W8f = np.asarray(W8, dtype=np.float32) / S_W
    dW = Wf - W8f
    # wt[h, kp, p, j, e, o5] = W8[e, 512h+o5, (2kp+j)*128+p]
    wt = np.ascontiguousarray(
        W8.reshape(E, NH, 512, KP, 2, 128).transpose(1, 3, 5, 4, 0, 2)
    )
    gw8 = (gwf.T * S_GW).astype(FP8)                    # [D, E]
    dgw = gwf.T - np.asarray(gw8, dtype=np.float32) / S_GW
    dgw8 = (dgw * S_DGW).astype(FP8)
    Gcat = np.zeros((D, 2, GP), dtype=FP8)
    Gcat[:, 0, 0:E] = np.asarray(gw8)
    Gcat[:, 1, 0:E] = np.asarray(dgw8)
    gs = np.ascontiguousarray(
        Gcat.reshape(KP, 2, 128, 2, GP).transpose(0, 2, 1, 3, 4)
    )
    gb = gbf.reshape(E, 1)
    cst = np.full((E, 1), 1.0 / 64.0, dtype=np.float32)
    ident = np.eye(E, dtype=np.float32).astype(BF16)

    # per-designated-expert correction matrices
    wcs = []
    for eh in range(E):
        m = np.full(E, m_off, dtype=np.float32)
        m[eh] = m_top
        A = np.einsum("e,eod->do", m, Wf)               # [D, O]
        Bm = np.einsum("e,eod->do", m, dW)
        A8 = (A * S_A).astype(FP8)
        B8 = (Bm * S_B).astype(FP8)
        # wc[h, k, p, j, o5]: j0 = A8 chunk, j1 = B8 chunk
        A8r = np.asarray(A8).reshape(KC, 128, NH, 512).transpose(2, 0, 1, 3)
        B8r = np.asarray(B8).reshape(KC, 128, NH, 512).transpose(2, 0, 1, 3)
        wcs.append(np.ascontiguousarray(np.stack([A8r, B8r], axis=3)))

    in_maps = []
    for c in range(N_CORES):
        idx = core_tokens[c]
        xc = x_flat[idx]                                # [T, D]
        x8 = xc.astype(FP8)
        dx = xc - np.asarray(x8, dtype=np.float32)
        dx8 = (dx * S_DX).astype(FP8)
        x8r = np.asarray(x8).T.reshape(KC, 128, T)
        dx8r = np.asarray(dx8).T.reshape(KC, 128, T)
        xi = np.ascontiguousarray(np.stack([dx8r, x8r], axis=2))
        in_maps.append(
            {
                "xi": xi,
                "wt": wt,
                "wc": wcs[c // 2],
                "gs": gs,
                "gb": gb,
                "cst": cst,
                "ident": ident,
            }
        )
    return in_maps, perm


def _run(inputs, trace=False):
    from concourse.bass_utils import run_bass_kernel_spmd

    if "nc" not in _CACHE:
        _CACHE["nc"] = _build_graph()
    nc = _CACHE["nc"]
    in_maps, perm = _prep_inputs(**inputs)
    res = run_bass_kernel_spmd(
        nc, in_maps, core_ids=list(range(N_CORES)), trace=trace
    )
    out = np.empty((N, O), dtype=np.float32)
    for c in range(N_CORES):
        shard = np.asarray(res.results[c]["out"], dtype=np.float32)
        out[perm[c * T : (c + 1) * T]] = shard
    return out.reshape(B, S, O), res


def kernel(x, W_experts, gate_w, gate_b):
    out, _ = _run(
        {"x": x, "W_experts": W_experts, "gate_w": gate_w, "gate_b": gate_b}
    )
    return out


# revision 9
# speedup vs baseline: 1.0163x; 1.0163x over previous
"""Trainium2 Bass kernel for AdaptiveProjection (dense MoE routing), fp8.

Computes: out[t,:] = sum_e softmax(x@gate_w.T + gate_b)[t,e] * (x[t] @ W_e.T)

Strategy:
- Data-parallel over tokens across 8 cores; weights replicated.
- Expert matmuls in fp8 e4m3 with DoubleRow perf mode (2x PE rate).
- Accuracy recovery (fp8 alone is ~3.2e-2 rel err, gate is 2e-2):
  * Tokens are routed (host-side permutation) so each core's tokens
    mostly share the same top-1 expert ("designated expert" = core//2).
  * One extra fp8 correction matmul per tile contracts the pair
    (dx8, x8) against (A, B), where dx8 = fp8 residual of x,
    A = sum_e m_e W_e (conditional-mean gates m given the designated
    top expert) and B = sum_e m_e (W_e - fp8(W_e)). This cancels the
    conditional-mean component of both x- and W-quantization errors.
  * Gate logits are computed from (x8 + dx8) @ (gw8 + dgw8) so gate
    precision stays bf16-class.
  Emulated end-to-end rel err: 1.2e-2.
"""

import numpy as np
import ml_dtypes

B, S, D, O, E = 4, 4096, 1024, 1024, 4
N = B * S
N_CORES = 8
T = N // N_CORES        # 2048 tokens per core
KC = D // 128           # 8 contraction chunks of 128
KP = KC // 2            # 4 DoubleRow k-pairs
NT = T // 128           # 16 token tiles per core
NH = O // 512           # 2 output halves
GBLK = 512              # gate-logit token block
GP = 64                 # padded gate-stationary columns (DoubleRow needs 32/64/128)

FP8 = ml_dtypes.float8_e4m3
BF16 = ml_dtypes.bfloat16
S_DX = 64.0             # dx8 = fp8(64*(x - x8))
S_W = 64.0              # W8 = fp8(64*W)
S_A = 8.0               # A8 = fp8(8*A)   -> corr j0 scale 64*8 = 512
S_B = 512.0             # B8 = fp8(512*B) -> corr j1 scale 512
S_GW = 8.0              # gw8 = fp8(8*gw)
S_DGW = 512.0           # dgw8 = fp8(512*(gw - gw8/8))

_CACHE = {}


def _build_graph():
    import concourse.mybir as mybir
    from concourse import bacc
    from concourse.bass import ts, ds
    from concourse.tile import TileContext

    f32 = mybir.dt.float32
    bf16 = mybir.dt.bfloat16
    fp8 = mybir.dt.float8e4
    DR = mybir.MatmulPerfMode.DoubleRow
    nc = bacc.Bacc(None, target_bir_lowering=False)

    xi_d = nc.declare_dram_parameter("xi", [KC, 128, 2, T], fp8, isOutput=False)
    wt_d = nc.declare_dram_parameter("wt", [NH, KP, 128, 2, E, 512], fp8, isOutput=False)
    wc_d = nc.declare_dram_parameter("wc", [NH, KC, 128, 2, 512], fp8, isOutput=False)
    gs_d = nc.declare_dram_parameter("gs", [KP, 128, 2, 2, GP], fp8, isOutput=False)
    gb_d = nc.declare_dram_parameter("gb", [E, 1], f32, isOutput=False)
    cst_d = nc.declare_dram_parameter("cst", [E, 1], f32, isOutput=False)
    id_d = nc.declare_dram_parameter("ident", [E, E], bf16, isOutput=False)
    out_d = nc.declare_dram_parameter("out", [T, O], bf16, isOutput=True)

    with TileContext(nc) as tc:
        with (
            tc.tile_pool(name="persist", bufs=1) as pp,
            tc.tile_pool(name="gate_sm", bufs=4) as gp,
            tc.tile_pool(name="acc", bufs=8) as ap,
        ):
            # --- persistent SBUF tensors ---
            xi_sb = pp.tile([128, KC, 2, T], fp8, tag="xi")
            w_sb = pp.tile([128, NH, KP, 2, E, 512], fp8, tag="w")
            wc_sb = pp.tile([128, NH, KC, 2, 512], fp8, tag="wc")
            gs_sb = pp.tile([128, KP, 2, 2, GP], fp8, tag="gs")
            gb_sb = pp.tile([E, 1], f32, tag="gb")
            cst_sb = pp.tile([E, 1], f32, tag="cst")
            id_sb = pp.tile([E, E], bf16, tag="ident")
            exp_sb = pp.tile([E, T], bf16, tag="exprow")
            gates_sb = pp.tile([128, NT * E], f32, tag="gates")

            # --- loads ---
            # Big streams ride the sync ring only; tiny tensors go on the
            # scalar ring early (ACT stays free later for the combine).
            scratch = pp.tile([128, 512], bf16, tag="scratch")
            nc.vector.memset(scratch[:, :], 0)
            nc.scalar.dma_start(out=gb_sb[:, :], in_=gb_d[:, :])
            nc.scalar.dma_start(out=cst_sb[:, :], in_=cst_d[:, :])
            nc.scalar.dma_start(out=id_sb[:, :], in_=id_d[:, :])
            for kp in range(KP):
                nc.sync.dma_start(out=gs_sb[:, kp, :, :, :], in_=gs_d[kp])
            for k in range(KC):
                nc.sync.dma_start(out=xi_sb[:, k, :, :], in_=xi_d[k])
            for k in range(KC):
                nc.sync.dma_start(out=wc_sb[:, 0, k, :, :], in_=wc_d[0, k])
            for kp in range(KP):
                nc.sync.dma_start(out=w_sb[:, 0, kp, :, :, :], in_=wt_d[0, kp])
            for k in range(KC):
                nc.sync.dma_start(out=wc_sb[:, 1, k, :, :], in_=wc_d[1, k])
            for kp in range(KP):
                nc.sync.dma_start(out=w_sb[:, 1, kp, :, :, :], in_=wt_d[1, kp])

            # --- gate prologue ---
            NB = T // GBLK
            with (
                tc.tile_pool(name="psum_w", bufs=1, space="PSUM") as pwp,
                tc.tile_pool(name="psum_g", bufs=2, space="PSUM") as pgp,
                tc.tile_pool(name="psum_t", bufs=1, space="PSUM") as ptp,
            ):
                warm_ps = pwp.tile([128, 512], f32, tag="warm")
                for _ in range(12):
                    nc.tensor.matmul(
                        warm_ps[:, :],
                        scratch[:, 0:128],
                        scratch[:, :],
                        start=True,
                        stop=True,
                        skip_group_check=True,
                    )
                # logits blocks: pa rows 0..3 = x8@gw8 (scale 8), rows 4..7 =
                # x8@dgw8 (scale 512); pb rows 0..3 = dx8@gw8 (scale 512).
                for wave in range(NB // 2):
                    blocks = (2 * wave, 2 * wave + 1)
                    pas = {b: pgp.tile([GP, GBLK], f32, tag="pa",
                                       name=f"pa{b}") for b in blocks}
                    pbs = {b: pgp.tile([GP, GBLK], f32, tag="pb",
                                       name=f"pb{b}") for b in blocks}
                    # pa = x8@gw8 (scale 8); pb = dx8@gw8 + x8@dgw8
                    # (both scale 512, one accumulation group)
                    for kp in range(KP):
                        for b in blocks:
                            nc.tensor.matmul(
                                pas[b][:, :],
                                gs_sb[:, kp, :, 0, :],
                                xi_sb[:, 2 * kp : 2 * kp + 2, 1, ts(b, GBLK)],
                                start=(kp == 0),
                                stop=(kp == KP - 1),
                                perf_mode=DR,
                            )
                        for b in blocks:
                            nc.tensor.matmul(
                                pbs[b][:, :],
                                gs_sb[:, kp, :, 0, :],
                                xi_sb[:, 2 * kp : 2 * kp + 2, 0, ts(b, GBLK)],
                                start=(kp == 0),
                                stop=False,
                                perf_mode=DR,
                            )
                    for kp in range(KP):
                        for b in blocks:
                            nc.tensor.matmul(
                                pbs[b][:, :],
                                gs_sb[:, kp, :, 1, :],
                                xi_sb[:, 2 * kp : 2 * kp + 2, 1, ts(b, GBLK)],
                                start=False,
                                stop=(kp == KP - 1),
                                perf_mode=DR,
                            )
                    # assemble logits and exp() per block
                    for b in blocks:
                        tb = gp.tile([E, GBLK], f32, tag="tb")
                        t1 = gp.tile([E, GBLK], f32, tag="t1")
                        nc.vector.tensor_copy(tb[:, :], pbs[b][0:E, :])
                        # t1 = (tb * 1/64) + pa[0:4]  (units of scale 8)
                        nc.vector.scalar_tensor_tensor(
                            t1[:, :],
                            tb[:, :],
                            cst_sb[:, 0:1],
                            pas[b][0:E, :],
                            op0=mybir.AluOpType.mult,
                            op1=mybir.AluOpType.add,
                        )
                        nc.scalar.activation(
                            exp_sb[:, ts(b, GBLK)],
                            t1[:, :],
                            mybir.ActivationFunctionType.Exp,
                            bias=gb_sb[:, 0:1],
                            scale=0.125,
                        )
                # transpose exp rows -> [128, E] per token tile
                expT = ptp.tile([128, NT * E], bf16, tag="expT")
                for t in range(NT):
                    nc.tensor.transpose(
                        expT[:, ts(t, E)],
                        exp_sb[:, ts(t, 128)],
                        id_sb[:, :],
                    )
                denom = gp.tile([128, NT], f32, tag="denom")
                recip = gp.tile([128, NT], f32, tag="recip")
                expT3 = expT[:, :].rearrange("p (t e) -> p t e", e=E)
                nc.vector.reduce_sum(
                    denom[:, :], expT3, axis=mybir.AxisListType.X
                )
                # recip = 1/(64*denom) so gates_sb holds g/64 (expert psums
                # carry the 64x weight scale)
                nc.scalar.activation(
                    denom[:, :],
                    denom[:, :],
                    mybir.ActivationFunctionType.Copy,
                    bias=0.0,
                    scale=64.0,
                )
                nc.vector.reciprocal(recip[:, :], denom[:, :])
                nc.vector.tensor_mul(
                    gates_sb[:, :].rearrange("p (t e) -> p t e", e=E),
                    expT3,
                    recip[:, :, None].broadcast_to([128, NT, E]),
                )

            # --- main loop: corr + expert matmuls, gated combine ---
            with (
                tc.tile_pool(name="psum_c", bufs=2, space="PSUM") as pcp,
                tc.tile_pool(name="psum_e", bufs=6, space="PSUM") as pep,
            ):
                for h in range(NH):
                    for t in range(NT):
                        pc = pcp.tile([128, 512], f32, tag="pc",
                                      name=f"pc{t}_{h}")
                        psums = [
                            pep.tile([128, 512], f32, tag="ep",
                                     name=f"ep{t}_{h}_{e}")
                            for e in range(E)
                        ]
                        # correction first so its bank drains early
                        for k in range(KC):
                            nc.tensor.matmul(
                                pc[:, :],
                                xi_sb[:, k, :, ts(t, 128)],
                                wc_sb[:, h, k, :, :],
                                start=(k == 0),
                                stop=(k == KC - 1),
                                perf_mode=DR,
                            )
                        for kp in range(KP):
                            lhs = xi_sb[:, 2 * kp : 2 * kp + 2, 1, ts(t, 128)]
                            for e in range(E):
                                nc.tensor.matmul(
                                    psums[e][:, :],
                                    lhs,
                                    w_sb[:, h, kp, :, e, :],
                                    start=(kp == 0),
                                    stop=(kp == KP - 1),
                                    perf_mode=DR,
                                )
                        # combine: out = sum_e g_e*p_e + pc/512
                        g0 = gates_sb[:, t * E + 0 : t * E + 1]
                        g1 = gates_sb[:, t * E + 1 : t * E + 2]
                        g2 = gates_sb[:, t * E + 2 : t * E + 3]
                        g3 = gates_sb[:, t * E + 3 : t * E + 4]
                        ca = ap.tile([128, 512], f32, tag="ca")
                        cb = ap.tile([128, 512], f32, tag="cb")
                        acc = ap.tile([128, 512], bf16, tag="acc")
                        nc.scalar.activation(
                            ca[:, :],
                            pc[:, :],
                            mybir.ActivationFunctionType.Copy,
                            bias=0.0,
                            scale=1.0 / 512.0,
                        )
                        nc.vector.scalar_tensor_tensor(
                            ca[:, :], psums[0][:, :], g0, ca[:, :],
                            op0=mybir.AluOpType.mult,
                            op1=mybir.AluOpType.add,
                        )
                        nc.scalar.activation(
                            cb[:, :],
                            psums[1][:, :],
                            mybir.ActivationFunctionType.Copy,
                            bias=0.0,
                            scale=g1,
                        )
                        nc.vector.scalar_tensor_tensor(
                            cb[:, :], psums[2][:, :], g2, cb[:, :],
                            op0=mybir.AluOpType.mult,
                            op1=mybir.AluOpType.add,
                        )
                        nc.vector.scalar_tensor_tensor(
                            ca[:, :], psums[3][:, :], g3, ca[:, :],
                            op0=mybir.AluOpType.mult,
                            op1=mybir.AluOpType.add,
                        )
                        nc.vector.tensor_add(acc[:, :], ca[:, :], cb[:, :])
                        nc.scalar.dma_start(
                            out=out_d[ts(t, 128), ds(512 * h, 512)],
                            in_=acc[:, :],
                        )
    nc.compile()
    return nc


def _prep_inputs(x, W_experts, gate_w, gate_b):
    x_flat = np.asarray(x, dtype=np.float32).reshape(N, D)
    Wf = np.asarray(W_experts, dtype=np.float32)        # [E, O, D]
    gwf = np.asarray(gate_w, dtype=np.float32)          # [E, D]
    gbf = np.asarray(gate_b, dtype=np.float32)          # [E]

    # host routing: top-1 expert per token (layout decision only)
    logits = x_flat @ gwf.T + gbf
    top = np.argmax(logits, -1)
    gh = np.exp(logits - logits.max(-1, keepdims=True))
    gh /= gh.sum(-1, keepdims=True)
    m_top = float(gh[np.arange(N), top].mean())
    m_off = (1.0 - m_top) / 3.0

    core_tokens = [None] * N_CORES
    spill = []
    for e in range(E):
        toks = np.where(top == e)[0]
        take = toks[: 2 * T]
        spill.append(toks[2 * T :])
        core_tokens[2 * e] = take[:T]
        core_tokens[2 * e + 1] = take[T : 2 * T]
    spill = np.concatenate(spill) if spill else np.empty(0, np.int64)
    for c in range(N_CORES):
        need = T - len(core_tokens[c])
        if need > 0:
            core_tokens[c] = np.concatenate([core_tokens[c], spill[:need]])
            spill = spill[need:]
    perm = np.concatenate(core_tokens)

    # weights (shared across cores)
    W8 = (Wf * S_W).astype(FP8)                         # [E, O, D]
    W8f = np.asarray(W8, dtype=np.float32) / S_W
    dW = Wf - W8f
    # wt[h, kp, p, j, e, o5] = W8[e, 512h+o5, (2kp+j)*128+p]
    wt = np.ascontiguousarray(
        W8.reshape(E, NH, 512, KP, 2, 128).transpose(1, 3, 5, 4, 0, 2)
    )
    gw8 = (gwf.T * S_GW).astype(FP8)                    # [D, E]
    dgw = gwf.T - np.asarray(gw8, dtype=np.float32) / S_GW
    dgw8 = (dgw * S_DGW).astype(FP8)
    Gcat = np.zeros((D, 2, GP), dtype=FP8)
    Gcat[:, 0, 0:E] = np.asarray(gw8)
    Gcat[:, 1, 0:E] = np.asarray(dgw8)
    gs = np.ascontiguousarray(
        Gcat.reshape(KP, 2, 128, 2, GP).transpose(0, 2, 1, 3, 4)
    )
    gb = gbf.reshape(E, 1)
    cst = np.full((E, 1), 1.0 / 64.0, dtype=np.float32)
    ident = np.eye(E, dtype=np.float32).astype(BF16)

    # per-designated-expert correction matrices
    wcs = []
    for eh in range(E):
        m = np.full(E, m_off, dtype=np.float32)
        m[eh] = m_top
        A = np.einsum("e,eod->do", m, Wf)               # [D, O]
        Bm = np.einsum("e,eod->do", m, dW)
        A8 = (A * S_A).astype(FP8)
        B8 = (Bm * S_B).astype(FP8)
        # wc[h, k, p, j, o5]: j0 = A8 chunk, j1 = B8 chunk
        A8r = np.asarray(A8).reshape(KC, 128, NH, 512).transpose(2, 0, 1, 3)
        B8r = np.asarray(B8).reshape(KC, 128, NH, 512).transpose(2, 0, 1, 3)
        wcs.append(np.ascontiguousarray(np.stack([A8r, B8r], axis=3)))

    in_maps = []
    for c in range(N_CORES):
        idx = core_tokens[c]
        xc = x_flat[idx]                                # [T, D]
        x8 = xc.astype(FP8)
        dx = xc - np.asarray(x8, dtype=np.float32)
        dx8 = (dx * S_DX).astype(FP8)
        x8r = np.asarray(x8).T.reshape(KC, 128, T)
        dx8r = np.asarray(dx8).T.reshape(KC, 128, T)
        xi = np.ascontiguousarray(np.stack([dx8r, x8r], axis=2))
        in_maps.append(
            {
                "xi": xi,
                "wt": wt,
                "wc": wcs[c // 2],
                "gs": gs,
                "gb": gb,
                "cst": cst,
                "ident": ident,
            }
        )
    return in_maps, perm


def _run(inputs, trace=False):
    from concourse.bass_utils import run_bass_kernel_spmd

    if "nc" not in _CACHE:
        _CACHE["nc"] = _build_graph()
    nc = _CACHE["nc"]
    in_maps, perm = _prep_inputs(**inputs)
    res = run_bass_kernel_spmd(
        nc, in_maps, core_ids=list(range(N_CORES)), trace=trace
    )
    out = np.empty((N, O), dtype=np.float32)
    for c in range(N_CORES):
        shard = np.asarray(res.results[c]["out"], dtype=np.float32)
        out[perm[c * T : (c + 1) * T]] = shard
    return out.reshape(B, S, O), res


def kernel(x, W_experts, gate_w, gate_b):
    out, _ = _run(
        {"x": x, "W_experts": W_experts, "gate_w": gate_w, "gate_b": gate_b}
    )
    return out


# revision 10
# speedup vs baseline: 1.0165x; 1.0002x over previous
"""Trainium2 Bass kernel for AdaptiveProjection (dense MoE routing), fp8.

Computes: out[t,:] = sum_e softmax(x@gate_w.T + gate_b)[t,e] * (x[t] @ W_e.T)

Strategy:
- Data-parallel over tokens across 8 cores; weights replicated.
- Expert matmuls in fp8 e4m3 with DoubleRow perf mode (2x PE rate).
- Accuracy recovery (fp8 alone is ~3.2e-2 rel err, gate is 2e-2):
  * Tokens are routed (host-side permutation) so each core's tokens
    mostly share the same top-1 expert ("designated expert" = core//2).
  * One extra fp8 correction matmul per tile contracts the pair
    (dx8, x8) against (A, B), where dx8 = fp8 residual of x,
    A = sum_e m_e W_e (conditional-mean gates m given the designated
    top expert) and B = sum_e m_e (W_e - fp8(W_e)). This cancels the
    conditional-mean component of both x- and W-quantization errors.
  * Gate logits are computed from (x8 + dx8) @ (gw8 + dgw8) so gate
    precision stays bf16-class.
  Emulated end-to-end rel err: 1.2e-2.
"""

import numpy as np
import ml_dtypes

B, S, D, O, E = 4, 4096, 1024, 1024, 4
N = B * S
N_CORES = 8
T = N // N_CORES        # 2048 tokens per core
KC = D // 128           # 8 contraction chunks of 128
KP = KC // 2            # 4 DoubleRow k-pairs
NT = T // 128           # 16 token tiles per core
NH = O // 512           # 2 output halves
GBLK = 512              # gate-logit token block
GP = 64                 # padded gate-stationary columns (DoubleRow needs 32/64/128)

FP8 = ml_dtypes.float8_e4m3
BF16 = ml_dtypes.bfloat16
S_DX = 64.0             # dx8 = fp8(64*(x - x8))
S_W = 64.0              # W8 = fp8(64*W)
S_A = 8.0               # A8 = fp8(8*A)   -> corr j0 scale 64*8 = 512
S_B = 512.0             # B8 = fp8(512*B) -> corr j1 scale 512
S_GW = 8.0              # gw8 = fp8(8*gw)
S_DGW = 512.0           # dgw8 = fp8(512*(gw - gw8/8))

_CACHE = {}


def _build_graph():
    import concourse.mybir as mybir
    from concourse import bacc
    from concourse.bass import ts, ds
    from concourse.tile import TileContext

    f32 = mybir.dt.float32
    bf16 = mybir.dt.bfloat16
    fp8 = mybir.dt.float8e4
    DR = mybir.MatmulPerfMode.DoubleRow
    nc = bacc.Bacc(None, target_bir_lowering=False)

    xi_d = nc.declare_dram_parameter("xi", [KC, 128, 2, T], fp8, isOutput=False)
    wt_d = nc.declare_dram_parameter("wt", [NH, KP, 128, 2, E, 512], fp8, isOutput=False)
    wc_d = nc.declare_dram_parameter("wc", [NH, KC, 128, 2, 512], fp8, isOutput=False)
    gs_d = nc.declare_dram_parameter("gs", [KP, 128, 2, 2, GP], fp8, isOutput=False)
    gb_d = nc.declare_dram_parameter("gb", [E, 1], f32, isOutput=False)
    cst_d = nc.declare_dram_parameter("cst", [E, 1], f32, isOutput=False)
    id_d = nc.declare_dram_parameter("ident", [E, E], bf16, isOutput=False)
    out_d = nc.declare_dram_parameter("out", [T, O], bf16, isOutput=True)

    with TileContext(nc) as tc:
        with (
            tc.tile_pool(name="persist", bufs=1) as pp,
            tc.tile_pool(name="gate_sm", bufs=4) as gp,
            tc.tile_pool(name="acc", bufs=8) as ap,
        ):
            # --- persistent SBUF tensors ---
            xi_sb = pp.tile([128, KC, 2, T], fp8, tag="xi")
            w_sb = pp.tile([128, NH, KP, 2, E, 512], fp8, tag="w")
            wc_sb = pp.tile([128, NH, KC, 2, 512], fp8, tag="wc")
            gs_sb = pp.tile([128, KP, 2, 2, GP], fp8, tag="gs")
            gb_sb = pp.tile([E, 1], f32, tag="gb")
            cst_sb = pp.tile([E, 1], f32, tag="cst")
            id_sb = pp.tile([E, E], bf16, tag="ident")
            exp_sb = pp.tile([E, T], bf16, tag="exprow")
            gates_sb = pp.tile([128, NT * E], f32, tag="gates")

            # --- loads ---
            # Big streams ride the sync ring only; tiny tensors go on the
            # scalar ring early (ACT stays free later for the combine).
            scratch = pp.tile([128, 512], bf16, tag="scratch")
            nc.gpsimd.memset(scratch[:, :], 0)
            nc.scalar.dma_start(out=gb_sb[:, :], in_=gb_d[:, :])
            nc.scalar.dma_start(out=cst_sb[:, :], in_=cst_d[:, :])
            nc.scalar.dma_start(out=id_sb[:, :], in_=id_d[:, :])
            for kp in range(KP):
                nc.sync.dma_start(out=gs_sb[:, kp, :, :, :], in_=gs_d[kp])
            for k in range(KC):
                nc.sync.dma_start(out=xi_sb[:, k, :, :], in_=xi_d[k])
            for k in range(KC):
                nc.sync.dma_start(out=wc_sb[:, 0, k, :, :], in_=wc_d[0, k])
            for kp in range(KP):
                nc.sync.dma_start(out=w_sb[:, 0, kp, :, :, :], in_=wt_d[0, kp])
            for k in range(KC):
                nc.sync.dma_start(out=wc_sb[:, 1, k, :, :], in_=wc_d[1, k])
            for kp in range(KP):
                nc.sync.dma_start(out=w_sb[:, 1, kp, :, :, :], in_=wt_d[1, kp])

            # --- gate prologue ---
            NB = T // GBLK
            with (
                tc.tile_pool(name="psum_w", bufs=1, space="PSUM") as pwp,
                tc.tile_pool(name="psum_g", bufs=2, space="PSUM") as pgp,
                tc.tile_pool(name="psum_t", bufs=1, space="PSUM") as ptp,
            ):
                warm_ps = pwp.tile([128, 512], f32, tag="warm")
                for _ in range(12):
                    nc.tensor.matmul(
                        warm_ps[:, :],
                        scratch[:, 0:128],
                        scratch[:, :],
                        start=True,
                        stop=True,
                        skip_group_check=True,
                    )
                # logits blocks: pa rows 0..3 = x8@gw8 (scale 8), rows 4..7 =
                # x8@dgw8 (scale 512); pb rows 0..3 = dx8@gw8 (scale 512).
                for wave in range(NB // 2):
                    blocks = (2 * wave, 2 * wave + 1)
                    pas = {b: pgp.tile([GP, GBLK], f32, tag="pa",
                                       name=f"pa{b}") for b in blocks}
                    pbs = {b: pgp.tile([GP, GBLK], f32, tag="pb",
                                       name=f"pb{b}") for b in blocks}
                    # pa = x8@gw8 (scale 8); pb = dx8@gw8 + x8@dgw8
                    # (both scale 512, one accumulation group)
                    for kp in range(KP):
                        for b in blocks:
                            nc.tensor.matmul(
                                pas[b][:, :],
                                gs_sb[:, kp, :, 0, :],
                                xi_sb[:, 2 * kp : 2 * kp + 2, 1, ts(b, GBLK)],
                                start=(kp == 0),
                                stop=(kp == KP - 1),
                                perf_mode=DR,
                            )
                        for b in blocks:
                            nc.tensor.matmul(
                                pbs[b][:, :],
                                gs_sb[:, kp, :, 0, :],
                                xi_sb[:, 2 * kp : 2 * kp + 2, 0, ts(b, GBLK)],
                                start=(kp == 0),
                                stop=False,
                                perf_mode=DR,
                            )
                    for kp in range(KP):
                        for b in blocks:
                            nc.tensor.matmul(
                                pbs[b][:, :],
                                gs_sb[:, kp, :, 1, :],
                                xi_sb[:, 2 * kp : 2 * kp + 2, 1, ts(b, GBLK)],
                                start=False,
                                stop=(kp == KP - 1),
                                perf_mode=DR,
                            )
                    # assemble logits and exp() per block
                    for b in blocks:
                        tb = gp.tile([E, GBLK], f32, tag="tb")
                        t1 = gp.tile([E, GBLK], f32, tag="t1")
                        nc.vector.tensor_copy(tb[:, :], pbs[b][0:E, :])
                        # t1 = (tb * 1/64) + pa[0:4]  (units of scale 8)
                        nc.vector.scalar_tensor_tensor(
                            t1[:, :],
                            tb[:, :],
                            cst_sb[:, 0:1],
                            pas[b][0:E, :],
                            op0=mybir.AluOpType.mult,
                            op1=mybir.AluOpType.add,
                        )
                        nc.scalar.activation(
                            exp_sb[:, ts(b, GBLK)],
                            t1[:, :],
                            mybir.ActivationFunctionType.Exp,
                            bias=gb_sb[:, 0:1],
                            scale=0.125,
                        )
                # transpose exp rows -> [128, E] per token tile
                expT = ptp.tile([128, NT * E], bf16, tag="expT")
                for t in range(NT):
                    nc.tensor.transpose(
                        expT[:, ts(t, E)],
                        exp_sb[:, ts(t, 128)],
                        id_sb[:, :],
                    )
                denom = gp.tile([128, NT], f32, tag="denom")
                recip = gp.tile([128, NT], f32, tag="recip")
                expT3 = expT[:, :].rearrange("p (t e) -> p t e", e=E)
                nc.vector.reduce_sum(
                    denom[:, :], expT3, axis=mybir.AxisListType.X
                )
                # recip = 1/(64*denom) so gates_sb holds g/64 (expert psums
                # carry the 64x weight scale)
                nc.scalar.activation(
                    denom[:, :],
                    denom[:, :],
                    mybir.ActivationFunctionType.Copy,
                    bias=0.0,
                    scale=64.0,
                )
                nc.vector.reciprocal(recip[:, :], denom[:, :])
                nc.vector.tensor_mul(
                    gates_sb[:, :].rearrange("p (t e) -> p t e", e=E),
                    expT3,
                    recip[:, :, None].broadcast_to([128, NT, E]),
                )

            # --- main loop: corr + expert matmuls, gated combine ---
            with (
                tc.tile_pool(name="psum_c", bufs=2, space="PSUM") as pcp,
                tc.tile_pool(name="psum_e", bufs=6, space="PSUM") as pep,
            ):
                for h in range(NH):
                    for t in range(NT):
                        pc = pcp.tile([128, 512], f32, tag="pc",
                                      name=f"pc{t}_{h}")
                        psums = [
                            pep.tile([128, 512], f32, tag="ep",
                                     name=f"ep{t}_{h}_{e}")
                            for e in range(E)
                        ]
                        # correction first so its bank drains early
                        for k in range(KC):
                            nc.tensor.matmul(
                                pc[:, :],
                                xi_sb[:, k, :, ts(t, 128)],
                                wc_sb[:, h, k, :, :],
                                start=(k == 0),
                                stop=(k == KC - 1),
                                perf_mode=DR,
                            )
                        for kp in range(KP):
                            lhs = xi_sb[:, 2 * kp : 2 * kp + 2, 1, ts(t, 128)]
                            for e in range(E):
                                nc.tensor.matmul(
                                    psums[e][:, :],
                                    lhs,
                                    w_sb[:, h, kp, :, e, :],
                                    start=(kp == 0),
                                    stop=(kp == KP - 1),
                                    perf_mode=DR,
                                )
                        # combine: out = sum_e g_e*p_e + pc/512
                        g0 = gates_sb[:, t * E + 0 : t * E + 1]
                        g1 = gates_sb[:, t * E + 1 : t * E + 2]
                        g2 = gates_sb[:, t * E + 2 : t * E + 3]
                        g3 = gates_sb[:, t * E + 3 : t * E + 4]
                        ca = ap.tile([128, 512], f32, tag="ca")
                        cb = ap.tile([128, 512], f32, tag="cb")
                        acc = ap.tile([128, 512], bf16, tag="acc")
                        nc.scalar.activation(
                            ca[:, :],
                            pc[:, :],
                            mybir.ActivationFunctionType.Copy,
                            bias=0.0,
                            scale=1.0 / 512.0,
                        )
                        nc.vector.scalar_tensor_tensor(
                            ca[:, :], psums[0][:, :], g0, ca[:, :],
                            op0=mybir.AluOpType.mult,
                            op1=mybir.AluOpType.add,
                        )
                        nc.scalar.activation(
                            cb[:, :],
                            psums[1][:, :],
                            mybir.ActivationFunctionType.Copy,
                            bias=0.0,
                            scale=g1,
                        )
                        nc.vector.scalar_tensor_tensor(
                            cb[:, :], psums[2][:, :], g2, cb[:, :],
                            op0=mybir.AluOpType.mult,
                            op1=mybir.AluOpType.add,
                        )
                        nc.vector.scalar_tensor_tensor(
                            ca[:, :], psums[3][:, :], g3, ca[:, :],
                            op0=mybir.AluOpType.mult,
                            op1=mybir.AluOpType.add,
                        )
                        nc.gpsimd.tensor_add(acc[:, :], ca[:, :], cb[:, :])
                        nc.sync.dma_start(
                            out=out_d[ts(t, 128), ds(512 * h, 512)],
                            in_=acc[:, :],
                        )
    nc.compile()
    return nc


def _prep_inputs(x, W_experts, gate_w, gate_b):
    x_flat = np.asarray(x, dtype=np.float32).reshape(N, D)
    Wf = np.asarray(W_experts, dtype=np.float32)        # [E, O, D]
    gwf = np.asarray(gate_w, dtype=np.float32)          # [E, D]
    gbf = np.asarray(gate_b, dtype=np.float32)          # [E]

    # host routing: top-1 expert per token (layout decision only)
    logits = x_flat @ gwf.T + gbf
    top = np.argmax(logits, -1)
    gh = np.exp(logits - logits.max(-1, keepdims=True))
    gh /= gh.sum(-1, keepdims=True)
    m_top = float(gh[np.arange(N), top].mean())
    m_off = (1.0 - m_top) / 3.0

    core_tokens = [None] * N_CORES
    spill = []
    for e in range(E):
        toks = np.where(top == e)[0]
        take = toks[: 2 * T]
        spill.append(toks[2 * T :])
        core_tokens[2 * e] = take[:T]
        core_tokens[2 * e + 1] = take[T : 2 * T]
    spill = np.concatenate(spill) if spill else np.empty(0, np.int64)
    for c in range(N_CORES):
        need = T - len(core_tokens[c])
        if need > 0:
            core_tokens[c] = np.concatenate([core_tokens[c], spill[:need]])
            spill = spill[need:]
    perm = np.concatenate(core_tokens)

    # weights (shared across cores)
    W8 = (Wf * S_W).astype(FP8)                         # [E, O, D]
    W8f = np.asarray(W8, dtype=np.float32) / S_W
    dW = Wf - W8f
    # wt[h, kp, p, j, e, o5] = W8[e, 512h+o5, (2kp+j)*128+p]
    wt = np.ascontiguousarray(
        W8.reshape(E, NH, 512, KP, 2, 128).transpose(1, 3, 5, 4, 0, 2)
    )
    gw8 = (gwf.T * S_GW).astype(FP8)                    # [D, E]
    dgw = gwf.T - np.asarray(gw8, dtype=np.float32) / S_GW
    dgw8 = (dgw * S_DGW).astype(FP8)
    Gcat = np.zeros((D, 2, GP), dtype=FP8)
    Gcat[:, 0, 0:E] = np.asarray(gw8)
    Gcat[:, 1, 0:E] = np.asarray(dgw8)
    gs = np.ascontiguousarray(
        Gcat.reshape(KP, 2, 128, 2, GP).transpose(0, 2, 1, 3, 4)
    )
    gb = gbf.reshape(E, 1)
    cst = np.full((E, 1), 1.0 / 64.0, dtype=np.float32)
    ident = np.eye(E, dtype=np.float32).astype(BF16)

    # per-designated-expert correction matrices
    wcs = []
    for eh in range(E):
        m = np.full(E, m_off, dtype=np.float32)
        m[eh] = m_top
        A = np.einsum("e,eod->do", m, Wf)               # [D, O]
        Bm = np.einsum("e,eod->do", m, dW)
        A8 = (A * S_A).astype(FP8)
        B8 = (Bm * S_B).astype(FP8)
        # wc[h, k, p, j, o5]: j0 = A8 chunk, j1 = B8 chunk
        A8r = np.asarray(A8).reshape(KC, 128, NH, 512).transpose(2, 0, 1, 3)
        B8r = np.asarray(B8).reshape(KC, 128, NH, 512).transpose(2, 0, 1, 3)
        wcs.append(np.ascontiguousarray(np.stack([A8r, B8r], axis=3)))

    in_maps = []
    for c in range(N_CORES):
        idx = core_tokens[c]
        xc = x_flat[idx]                                # [T, D]
        x8 = xc.astype(FP8)
        dx = xc - np.asarray(x8, dtype=np.float32)
        dx8 = (dx * S_DX).astype(FP8)
        x8r = np.asarray(x8).T.reshape(KC, 128, T)
        dx8r = np.asarray(dx8).T.reshape(KC, 128, T)
        xi = np.ascontiguousarray(np.stack([dx8r, x8r], axis=2))
        in_maps.append(
            {
                "xi": xi,
                "wt": wt,
                "wc": wcs[c // 2],
                "gs": gs,
                "gb": gb,
                "cst": cst,
                "ident": ident,
            }
        )
    return in_maps, perm


def _run(inputs, trace=False):
    from concourse.bass_utils import run_bass_kernel_spmd

    if "nc" not in _CACHE:
        _CACHE["nc"] = _build_graph()
    nc = _CACHE["nc"]
    in_maps, perm = _prep_inputs(**inputs)
    res = run_bass_kernel_spmd(
        nc, in_maps, core_ids=list(range(N_CORES)), trace=trace
    )
    out = np.empty((N, O), dtype=np.float32)
    for c in range(N_CORES):
        shard = np.asarray(res.results[c]["out"], dtype=np.float32)
        out[perm[c * T : (c + 1) * T]] = shard
    return out.reshape(B, S, O), res


def kernel(x, W_experts, gate_w, gate_b):
    out, _ = _run(
        {"x": x, "W_experts": W_experts, "gate_w": gate_w, "gate_b": gate_b}
    )
    return out


# revision 11
# speedup vs baseline: 1.0370x; 1.0202x over previous
"""Trainium2 Bass kernel for AdaptiveProjection (dense MoE routing), fp8.

Computes: out[t,:] = sum_e softmax(x@gate_w.T + gate_b)[t,e] * (x[t] @ W_e.T)

Strategy:
- Data-parallel over tokens across 8 cores; weights replicated.
- Expert matmuls in fp8 e4m3 with DoubleRow perf mode (2x PE rate).
- Accuracy recovery (fp8 alone is ~3.2e-2 rel err, gate is 2e-2):
  * Tokens are routed (host-side permutation) so each core's tokens
    mostly share the same top-1 expert ("designated expert" = core//2).
  * One extra fp8 correction matmul per tile contracts the pair
    (dx8, x8) against (A, B), where dx8 = fp8 residual of x,
    A = sum_e m_e W_e (conditional-mean gates m given the designated
    top expert) and B = sum_e m_e (W_e - fp8(W_e)). This cancels the
    conditional-mean component of both x- and W-quantization errors.
  * Gate logits are computed from (x8 + dx8) @ (gw8 + dgw8) so gate
    precision stays bf16-class.
  Emulated end-to-end rel err: 1.2e-2.
"""

import numpy as np
import ml_dtypes

B, S, D, O, E = 4, 4096, 1024, 1024, 4
N = B * S
N_CORES = 8
T = N // N_CORES        # 2048 tokens per core
KC = D // 128           # 8 contraction chunks of 128
KP = KC // 2            # 4 DoubleRow k-pairs
NT = T // 128           # 16 token tiles per core
NH = O // 512           # 2 output halves
GBLK = 512              # gate-logit token block
GP = 64                 # padded gate-stationary columns (DoubleRow needs 32/64/128)

FP8 = ml_dtypes.float8_e4m3
BF16 = ml_dtypes.bfloat16
S_DX = 64.0             # dx8 = fp8(64*(x - x8))
S_W = 64.0              # W8 = fp8(64*W)
S_A = 8.0               # A8 = fp8(8*A)   -> corr j0 scale 64*8 = 512
S_B = 512.0             # B8 = fp8(512*B) -> corr j1 scale 512
S_GW = 8.0              # gw8 = fp8(8*gw)
S_DGW = 512.0           # dgw8 = fp8(512*(gw - gw8/8))

_CACHE = {}


def _build_graph():
    import concourse.mybir as mybir
    from concourse import bacc
    from concourse.bass import ts, ds
    from concourse.tile import TileContext

    f32 = mybir.dt.float32
    bf16 = mybir.dt.bfloat16
    fp8 = mybir.dt.float8e4
    DR = mybir.MatmulPerfMode.DoubleRow
    nc = bacc.Bacc(None, target_bir_lowering=False)

    xi_d = nc.declare_dram_parameter("xi", [KC, 128, 2, T], fp8, isOutput=False)
    wt_d = nc.declare_dram_parameter("wt", [NH, KP, 128, 2, E, 512], fp8, isOutput=False)
    wc_d = nc.declare_dram_parameter("wc", [NH, KC, 128, 2, 512], fp8, isOutput=False)
    gs_d = nc.declare_dram_parameter("gs", [KP, 128, 2, 2, GP], fp8, isOutput=False)
    gb_d = nc.declare_dram_parameter("gb", [E, 1], f32, isOutput=False)
    cst_d = nc.declare_dram_parameter("cst", [E, 1], f32, isOutput=False)
    id_d = nc.declare_dram_parameter("ident", [E, E], bf16, isOutput=False)
    out_d = nc.declare_dram_parameter("out", [T, O], bf16, isOutput=True)

    with TileContext(nc) as tc:
        with (
            tc.tile_pool(name="persist", bufs=1) as pp,
            tc.tile_pool(name="gate_sm", bufs=4) as gp,
            tc.tile_pool(name="acc", bufs=8) as ap,
        ):
            # --- persistent SBUF tensors ---
            xi_sb = pp.tile([128, KC, 2, T], fp8, tag="xi")
            w_sb = pp.tile([128, NH, KP, 2, E, 512], fp8, tag="w")
            wc_sb = pp.tile([128, NH, KC, 2, 512], fp8, tag="wc")
            gs_sb = pp.tile([128, KP, 2, 2, GP], fp8, tag="gs")
            gb_sb = pp.tile([E, 1], f32, tag="gb")
            cst_sb = pp.tile([E, 1], f32, tag="cst")
            id_sb = pp.tile([E, E], bf16, tag="ident")
            exp_sb = pp.tile([E, T], bf16, tag="exprow")
            gates_sb = pp.tile([128, NT * E], f32, tag="gates")

            # --- loads ---
            # Big streams ride the sync ring only; tiny tensors go on the
            # scalar ring early (ACT stays free later for the combine).
            scratch = pp.tile([128, 512], bf16, tag="scratch")
            nc.gpsimd.memset(scratch[:, :], 0)
            nc.scalar.dma_start(out=gb_sb[:, :], in_=gb_d[:, :])
            nc.scalar.dma_start(out=cst_sb[:, :], in_=cst_d[:, :])
            nc.scalar.dma_start(out=id_sb[:, :], in_=id_d[:, :])
            for kp in range(KP):
                nc.sync.dma_start(out=gs_sb[:, kp, :, :, :], in_=gs_d[kp])
            for k in range(KC):
                nc.sync.dma_start(out=xi_sb[:, k, :, :], in_=xi_d[k])
            for kp in range(KP):
                nc.sync.dma_start(out=w_sb[:, 0, kp, :, :, :], in_=wt_d[0, kp])
            for k in range(KC):
                nc.sync.dma_start(out=wc_sb[:, 0, k, :, :], in_=wc_d[0, k])
            for k in range(KC):
                nc.sync.dma_start(out=wc_sb[:, 1, k, :, :], in_=wc_d[1, k])
            for kp in range(KP):
                nc.sync.dma_start(out=w_sb[:, 1, kp, :, :, :], in_=wt_d[1, kp])

            # --- gate prologue ---
            NB = T // GBLK
            with (
                tc.tile_pool(name="psum_g", bufs=4, space="PSUM") as pgp,
            ):
                # logits blocks: pa = x8@gw8 (scale 8); pb = dx8@gw8 +
                # x8@dgw8 (both scale 512, one accumulation group).
                # kp-outer with all three passes per k-pair keeps the PE
                # paced exactly with the arriving xi DMA stream.
                pas = {b: pgp.tile([GP, GBLK], f32, tag="pa",
                                   name=f"pa{b}") for b in range(NB)}
                pbs = {b: pgp.tile([GP, GBLK], f32, tag="pb",
                                   name=f"pb{b}") for b in range(NB)}
                # warm-up: ramp the PE clock on scratch before real work
                for _ in range(12):
                    nc.tensor.matmul(
                        pas[0][:, :],
                        scratch[:, 0:GP],
                        scratch[:, :],
                        start=True,
                        stop=True,
                        skip_group_check=True,
                    )
                for kp in range(KP):
                    for b in range(NB):
                        nc.tensor.matmul(
                            pas[b][:, :],
                            gs_sb[:, kp, :, 0, :],
                            xi_sb[:, 2 * kp : 2 * kp + 2, 1, ts(b, GBLK)],
                            start=(kp == 0),
                            stop=(kp == KP - 1),
                            perf_mode=DR,
                        )
                    for b in range(NB):
                        nc.tensor.matmul(
                            pbs[b][:, :],
                            gs_sb[:, kp, :, 0, :],
                            xi_sb[:, 2 * kp : 2 * kp + 2, 0, ts(b, GBLK)],
                            start=(kp == 0),
                            stop=False,
                            perf_mode=DR,
                        )
                    for b in range(NB):
                        nc.tensor.matmul(
                            pbs[b][:, :],
                            gs_sb[:, kp, :, 1, :],
                            xi_sb[:, 2 * kp : 2 * kp + 2, 1, ts(b, GBLK)],
                            start=False,
                            stop=(kp == KP - 1),
                            perf_mode=DR,
                        )
                if True:
                    # assemble logits and exp() per block
                    for b in range(NB):
                        tb = gp.tile([E, GBLK], f32, tag="tb")
                        t1 = gp.tile([E, GBLK], f32, tag="t1")
                        nc.vector.tensor_copy(tb[:, :], pbs[b][0:E, :])
                        # t1 = (tb * 1/64) + pa[0:4]  (units of scale 8)
                        nc.vector.scalar_tensor_tensor(
                            t1[:, :],
                            tb[:, :],
                            cst_sb[:, 0:1],
                            pas[b][0:E, :],
                            op0=mybir.AluOpType.mult,
                            op1=mybir.AluOpType.add,
                        )
                        nc.scalar.activation(
                            exp_sb[:, ts(b, GBLK)],
                            t1[:, :],
                            mybir.ActivationFunctionType.Exp,
                            bias=gb_sb[:, 0:1],
                            scale=0.125,
                        )
            with tc.tile_pool(name="psum_t", bufs=1, space="PSUM") as ptp:
                # transpose exp rows -> [128, E] per token tile
                expT = ptp.tile([128, NT * E], bf16, tag="expT")
                for t in range(NT):
                    nc.tensor.transpose(
                        expT[:, ts(t, E)],
                        exp_sb[:, ts(t, 128)],
                        id_sb[:, :],
                    )
                denom = gp.tile([128, NT], f32, tag="denom")
                recip = gp.tile([128, NT], f32, tag="recip")
                expT3 = expT[:, :].rearrange("p (t e) -> p t e", e=E)
                nc.vector.reduce_sum(
                    denom[:, :], expT3, axis=mybir.AxisListType.X
                )
                # recip = 1/(64*denom) so gates_sb holds g/64 (expert psums
                # carry the 64x weight scale)
                nc.scalar.activation(
                    denom[:, :],
                    denom[:, :],
                    mybir.ActivationFunctionType.Copy,
                    bias=0.0,
                    scale=64.0,
                )
                nc.vector.reciprocal(recip[:, :], denom[:, :])
                nc.vector.tensor_mul(
                    gates_sb[:, :].rearrange("p (t e) -> p t e", e=E),
                    expT3,
                    recip[:, :, None].broadcast_to([128, NT, E]),
                )

            # --- main loop: corr + expert matmuls, gated combine ---
            with (
                tc.tile_pool(name="psum_c", bufs=2, space="PSUM") as pcp,
                tc.tile_pool(name="psum_e", bufs=6, space="PSUM") as pep,
            ):
                for h in range(NH):
                    for t in range(NT):
                        pc = pcp.tile([128, 512], f32, tag="pc",
                                      name=f"pc{t}_{h}")
                        psums = [
                            pep.tile([128, 512], f32, tag="ep",
                                     name=f"ep{t}_{h}_{e}")
                            for e in range(E)
                        ]

                        def corr_mms():
                            for k in range(KC):
                                nc.tensor.matmul(
                                    pc[:, :],
                                    xi_sb[:, k, :, ts(t, 128)],
                                    wc_sb[:, h, k, :, :],
                                    start=(k == 0),
                                    stop=(k == KC - 1),
                                    perf_mode=DR,
                                )

                        def expert_mms():
                            for kp in range(KP):
                                lhs = xi_sb[
                                    :, 2 * kp : 2 * kp + 2, 1, ts(t, 128)
                                ]
                                for e in range(E):
                                    nc.tensor.matmul(
                                        psums[e][:, :],
                                        lhs,
                                        w_sb[:, h, kp, :, e, :],
                                        start=(kp == 0),
                                        stop=(kp == KP - 1),
                                        perf_mode=DR,
                                    )

                        # correction first so its bank drains early; but the
                        # very first tile runs experts first (wt0 lands
                        # before wc0 in the DMA stream)
                        if h == 0 and t == 0:
                            expert_mms()
                            corr_mms()
                        else:
                            corr_mms()
                            expert_mms()
                        # combine: out = sum_e g_e*p_e + pc/512
                        g0 = gates_sb[:, t * E + 0 : t * E + 1]
                        g1 = gates_sb[:, t * E + 1 : t * E + 2]
                        g2 = gates_sb[:, t * E + 2 : t * E + 3]
                        g3 = gates_sb[:, t * E + 3 : t * E + 4]
                        ca = ap.tile([128, 512], f32, tag="ca")
                        cb = ap.tile([128, 512], f32, tag="cb")
                        acc = ap.tile([128, 512], bf16, tag="acc")
                        nc.scalar.activation(
                            ca[:, :],
                            pc[:, :],
                            mybir.ActivationFunctionType.Copy,
                            bias=0.0,
                            scale=1.0 / 512.0,
                        )
                        nc.vector.scalar_tensor_tensor(
                            ca[:, :], psums[0][:, :], g0, ca[:, :],
                            op0=mybir.AluOpType.mult,
                            op1=mybir.AluOpType.add,
                        )
                        nc.scalar.activation(
                            cb[:, :],
                            psums[1][:, :],
                            mybir.ActivationFunctionType.Copy,
                            bias=0.0,
                            scale=g1,
                        )
                        nc.vector.scalar_tensor_tensor(
                            cb[:, :], psums[2][:, :], g2, cb[:, :],
                            op0=mybir.AluOpType.mult,
                            op1=mybir.AluOpType.add,
                        )
                        nc.vector.scalar_tensor_tensor(
                            ca[:, :], psums[3][:, :], g3, ca[:, :],
                            op0=mybir.AluOpType.mult,
                            op1=mybir.AluOpType.add,
                        )
                        nc.vector.tensor_add(acc[:, :], ca[:, :], cb[:, :])
                        nc.sync.dma_start(
                            out=out_d[ts(t, 128), ds(512 * h, 512)],
                            in_=acc[:, :],
                        )
    nc.compile()
    return nc


def _prep_inputs(x, W_experts, gate_w, gate_b):
    x_flat = np.asarray(x, dtype=np.float32).reshape(N, D)
    Wf = np.asarray(W_experts, dtype=np.float32)        # [E, O, D]
    gwf = np.asarray(gate_w, dtype=np.float32)          # [E, D]
    gbf = np.asarray(gate_b, dtype=np.float32)          # [E]

    # host routing: top-1 expert per token (layout decision only)
    logits = x_flat @ gwf.T + gbf
    top = np.argmax(logits, -1)
    gh = np.exp(logits - logits.max(-1, keepdims=True))
    gh /= gh.sum(-1, keepdims=True)
    m_top = float(gh[np.arange(N), top].mean())
    m_off = (1.0 - m_top) / 3.0

    core_tokens = [None] * N_CORES
    spill = []
    for e in range(E):
        toks = np.where(top == e)[0]
        take = toks[: 2 * T]
        spill.append(toks[2 * T :])
        core_tokens[2 * e] = take[:T]
        core_tokens[2 * e + 1] = take[T : 2 * T]
    spill = np.concatenate(spill) if spill else np.empty(0, np.int64)
    for c in range(N_CORES):
        need = T - len(core_tokens[c])
        if need > 0:
            core_tokens[c] = np.concatenate([core_tokens[c], spill[:need]])
            spill = spill[need:]
    perm = np.concatenate(core_tokens)

    # weights (shared across cores)
    W8 = (Wf * S_W).astype(FP8)                         # [E, O, D]
    W8f = np.asarray(W8, dtype=np.float32) / S_W
    dW = Wf - W8f
    # wt[h, kp, p, j, e, o5] = W8[e, 512h+o5, (2kp+j)*128+p]
    wt = np.ascontiguousarray(
        W8.reshape(E, NH, 512, KP, 2, 128).transpose(1, 3, 5, 4, 0, 2)
    )
    gw8 = (gwf.T * S_GW).astype(FP8)                    # [D, E]
    dgw = gwf.T - np.asarray(gw8, dtype=np.float32) / S_GW
    dgw8 = (dgw * S_DGW).astype(FP8)
    Gcat = np.zeros((D, 2, GP), dtype=FP8)
    Gcat[:, 0, 0:E] = np.asarray(gw8)
    Gcat[:, 1, 0:E] = np.asarray(dgw8)
    gs = np.ascontiguousarray(
        Gcat.reshape(KP, 2, 128, 2, GP).transpose(0, 2, 1, 3, 4)
    )
    gb = gbf.reshape(E, 1)
    cst = np.full((E, 1), 1.0 / 64.0, dtype=np.float32)
    ident = np.eye(E, dtype=np.float32).astype(BF16)

    # per-designated-expert correction matrices
    wcs = []
    for eh in range(E):
        m = np.full(E, m_off, dtype=np.float32)
        m[eh] = m_top
        A = np.einsum("e,eod->do", m, Wf)               # [D, O]
        Bm = np.einsum("e,eod->do", m, dW)
        A8 = (A * S_A).astype(FP8)
        B8 = (Bm * S_B).astype(FP8)
        # wc[h, k, p, j, o5]: j0 = A8 chunk, j1 = B8 chunk
        A8r = np.asarray(A8).reshape(KC, 128, NH, 512).transpose(2, 0, 1, 3)
        B8r = np.asarray(B8).reshape(KC, 128, NH, 512).transpose(2, 0, 1, 3)
        wcs.append(np.ascontiguousarray(np.stack([A8r, B8r], axis=3)))

    in_maps = []
    for c in range(N_CORES):
        idx = core_tokens[c]
        xc = x_flat[idx]                                # [T, D]
        x8 = xc.astype(FP8)
        dx = xc - np.asarray(x8, dtype=np.float32)
        dx8 = (dx * S_DX).astype(FP8)
        x8r = np.asarray(x8).T.reshape(KC, 128, T)
        dx8r = np.asarray(dx8).T.reshape(KC, 128, T)
        xi = np.ascontiguousarray(np.stack([dx8r, x8r], axis=2))
        in_maps.append(
            {
                "xi": xi,
                "wt": wt,
                "wc": wcs[c // 2],
                "gs": gs,
                "gb": gb,
                "cst": cst,
                "ident": ident,
            }
        )
    return in_maps, perm


def _run(inputs, trace=False):
    from concourse.bass_utils import run_bass_kernel_spmd

    if "nc" not in _CACHE:
        _CACHE["nc"] = _build_graph()
    nc = _CACHE["nc"]
    in_maps, perm = _prep_inputs(**inputs)
    res = run_bass_kernel_spmd(
        nc, in_maps, core_ids=list(range(N_CORES)), trace=trace
    )
    out = np.empty((N, O), dtype=np.float32)
    for c in range(N_CORES):
        shard = np.asarray(res.results[c]["out"], dtype=np.float32)
        out[perm[c * T : (c + 1) * T]] = shard
    return out.reshape(B, S, O), res


def kernel(x, W_experts, gate_w, gate_b):
    out, _ = _run(
        {"x": x, "W_experts": W_experts, "gate_w": gate_w, "gate_b": gate_b}
    )
    return out


# revision 12
# speedup vs baseline: 1.0506x; 1.0130x over previous
"""Trainium2 Bass kernel for AdaptiveProjection (dense MoE routing), fp8.

Computes: out[t,:] = sum_e softmax(x@gate_w.T + gate_b)[t,e] * (x[t] @ W_e.T)

Strategy:
- Data-parallel over tokens across 8 cores; weights replicated.
- Expert matmuls in fp8 e4m3 with DoubleRow perf mode (2x PE rate).
- Accuracy recovery (fp8 alone is ~3.2e-2 rel err, gate is 2e-2):
  * Tokens are routed (host-side permutation) so each core's tokens
    mostly share the same top-1 expert ("designated expert" = core//2).
  * One extra fp8 correction matmul per tile contracts the pair
    (dx8, x8) against (A, B), where dx8 = fp8 residual of x,
    A = sum_e m_e W_e (conditional-mean gates m given the designated
    top expert) and B = sum_e m_e (W_e - fp8(W_e)). This cancels the
    conditional-mean component of both x- and W-quantization errors.
  * Gate logits are computed from (x8 + dx8) @ (gw8 + dgw8) so gate
    precision stays bf16-class.
  Emulated end-to-end rel err: 1.2e-2.
"""

import numpy as np
import ml_dtypes

B, S, D, O, E = 4, 4096, 1024, 1024, 4
N = B * S
N_CORES = 8
T = N // N_CORES        # 2048 tokens per core
KC = D // 128           # 8 contraction chunks of 128
KP = KC // 2            # 4 DoubleRow k-pairs
NT = T // 128           # 16 token tiles per core
NH = O // 512           # 2 output halves
GBLK = 512              # gate-logit token block
GP = 64                 # padded gate-stationary columns (DoubleRow needs 32/64/128)

FP8 = ml_dtypes.float8_e4m3
BF16 = ml_dtypes.bfloat16
S_DX = 64.0             # dx8 = fp8(64*(x - x8))
S_W = 64.0              # W8 = fp8(64*W)
S_A = 8.0               # A8 = fp8(8*A)   -> corr j0 scale 64*8 = 512
S_B = 512.0             # B8 = fp8(512*B) -> corr j1 scale 512
S_GW = 8.0              # gw8 = fp8(8*gw)
S_DGW = 512.0           # dgw8 = fp8(512*(gw - gw8/8))

_CACHE = {}


def _build_graph():
    import concourse.mybir as mybir
    from concourse import bacc
    from concourse.bass import ts, ds
    from concourse.tile import TileContext

    f32 = mybir.dt.float32
    bf16 = mybir.dt.bfloat16
    fp8 = mybir.dt.float8e4
    DR = mybir.MatmulPerfMode.DoubleRow
    nc = bacc.Bacc(None, target_bir_lowering=False)

    xi_d = nc.declare_dram_parameter("xi", [KC, 128, 2, T], fp8, isOutput=False)
    wt_d = nc.declare_dram_parameter("wt", [NH, KP, 128, 2, E, 512], fp8, isOutput=False)
    wc_d = nc.declare_dram_parameter("wc", [NH, KC, 128, 2, 512], fp8, isOutput=False)
    gs_d = nc.declare_dram_parameter("gs", [KP, 128, 2, 2, GP], fp8, isOutput=False)
    gb_d = nc.declare_dram_parameter("gb", [E, 1], f32, isOutput=False)
    cst_d = nc.declare_dram_parameter("cst", [E, 1], f32, isOutput=False)
    id_d = nc.declare_dram_parameter("ident", [E, E], bf16, isOutput=False)
    out_d = nc.declare_dram_parameter("out", [T, O], bf16, isOutput=True)

    with TileContext(nc) as tc:
        with (
            tc.tile_pool(name="persist", bufs=1) as pp,
            tc.tile_pool(name="gate_sm", bufs=4) as gp,
            tc.tile_pool(name="acc", bufs=8) as ap,
        ):
            # --- persistent SBUF tensors ---
            xi_sb = pp.tile([128, KC, 2, T], fp8, tag="xi")
            w_sb = pp.tile([128, NH, KP, 2, E, 512], fp8, tag="w")
            wc_sb = pp.tile([128, NH, KC, 2, 512], fp8, tag="wc")
            gs_sb = pp.tile([128, KP, 2, 2, GP], fp8, tag="gs")
            gb_sb = pp.tile([E, 1], f32, tag="gb")
            cst_sb = pp.tile([E, 1], f32, tag="cst")
            id_sb = pp.tile([E, E], bf16, tag="ident")
            exp_sb = pp.tile([E, T], bf16, tag="exprow")
            gates_sb = pp.tile([128, NT * E], f32, tag="gates")

            # --- loads ---
            # Big streams ride the sync ring only; tiny tensors go on the
            # scalar ring early (ACT stays free later for the combine).
            nc.scalar.dma_start(out=gb_sb[:, :], in_=gb_d[:, :])
            nc.scalar.dma_start(out=cst_sb[:, :], in_=cst_d[:, :])
            nc.scalar.dma_start(out=id_sb[:, :], in_=id_d[:, :])
            for kp in range(KP):
                nc.sync.dma_start(out=gs_sb[:, kp, :, :, :], in_=gs_d[kp])
            for k in range(KC):
                nc.sync.dma_start(out=xi_sb[:, k, :, :], in_=xi_d[k])
            for kp in range(KP):
                nc.sync.dma_start(out=w_sb[:, 0, kp, :, :, :], in_=wt_d[0, kp])
            for k in range(KC):
                nc.sync.dma_start(out=wc_sb[:, 0, k, :, :], in_=wc_d[0, k])
            for k in range(KC):
                nc.sync.dma_start(out=wc_sb[:, 1, k, :, :], in_=wc_d[1, k])
            for kp in range(KP):
                nc.sync.dma_start(out=w_sb[:, 1, kp, :, :, :], in_=wt_d[1, kp])

            # --- gate prologue ---
            NB = T // GBLK
            with (
                tc.tile_pool(name="psum_g", bufs=4, space="PSUM") as pgp,
            ):
                # logits blocks: pa = x8@gw8 (scale 8); pb = dx8@gw8 +
                # x8@dgw8 (both scale 512, one accumulation group).
                # kp-outer with all three passes per k-pair keeps the PE
                # paced exactly with the arriving xi DMA stream.
                pas = {b: pgp.tile([GP, GBLK], f32, tag="pa",
                                   name=f"pa{b}") for b in range(NB)}
                pbs = {b: pgp.tile([GP, GBLK], f32, tag="pb",
                                   name=f"pb{b}") for b in range(NB)}
                for kp in range(KP):
                    for b in range(NB):
                        nc.tensor.matmul(
                            pas[b][:, :],
                            gs_sb[:, kp, :, 0, :],
                            xi_sb[:, 2 * kp : 2 * kp + 2, 1, ts(b, GBLK)],
                            start=(kp == 0),
                            stop=(kp == KP - 1),
                            perf_mode=DR,
                        )
                        nc.tensor.matmul(
                            pbs[b][:, :],
                            gs_sb[:, kp, :, 0, :],
                            xi_sb[:, 2 * kp : 2 * kp + 2, 0, ts(b, GBLK)],
                            start=(kp == 0),
                            stop=False,
                            perf_mode=DR,
                        )
                        nc.tensor.matmul(
                            pbs[b][:, :],
                            gs_sb[:, kp, :, 1, :],
                            xi_sb[:, 2 * kp : 2 * kp + 2, 1, ts(b, GBLK)],
                            start=False,
                            stop=(kp == KP - 1),
                            perf_mode=DR,
                        )
                if True:
                    # assemble logits and exp() per block
                    for b in range(NB):
                        tb = gp.tile([E, GBLK], f32, tag="tb")
                        t1 = gp.tile([E, GBLK], f32, tag="t1")
                        nc.vector.tensor_copy(tb[:, :], pbs[b][0:E, :])
                        # t1 = (tb * 1/64) + pa[0:4]  (units of scale 8)
                        nc.vector.scalar_tensor_tensor(
                            t1[:, :],
                            tb[:, :],
                            cst_sb[:, 0:1],
                            pas[b][0:E, :],
                            op0=mybir.AluOpType.mult,
                            op1=mybir.AluOpType.add,
                        )
                        nc.scalar.activation(
                            exp_sb[:, ts(b, GBLK)],
                            t1[:, :],
                            mybir.ActivationFunctionType.Exp,
                            bias=gb_sb[:, 0:1],
                            scale=0.125,
                        )
            with tc.tile_pool(name="psum_t", bufs=1, space="PSUM") as ptp:
                # transpose exp rows -> [128, E] per token tile
                expT = ptp.tile([128, NT * E], bf16, tag="expT")
                for t in range(NT):
                    nc.tensor.transpose(
                        expT[:, ts(t, E)],
                        exp_sb[:, ts(t, 128)],
                        id_sb[:, :],
                    )
                denom = gp.tile([128, NT], f32, tag="denom")
                recip = gp.tile([128, NT], f32, tag="recip")
                expT3 = expT[:, :].rearrange("p (t e) -> p t e", e=E)
                nc.vector.reduce_sum(
                    denom[:, :], expT3, axis=mybir.AxisListType.X
                )
                # recip = 1/(64*denom) so gates_sb holds g/64 (expert psums
                # carry the 64x weight scale)
                nc.scalar.activation(
                    denom[:, :],
                    denom[:, :],
                    mybir.ActivationFunctionType.Copy,
                    bias=0.0,
                    scale=64.0,
                )
                nc.vector.reciprocal(recip[:, :], denom[:, :])
                nc.vector.tensor_mul(
                    gates_sb[:, :].rearrange("p (t e) -> p t e", e=E),
                    expT3,
                    recip[:, :, None].broadcast_to([128, NT, E]),
                )

            # --- main loop: corr + expert matmuls, gated combine ---
            with (
                tc.tile_pool(name="psum_c", bufs=2, space="PSUM") as pcp,
                tc.tile_pool(name="psum_e", bufs=6, space="PSUM") as pep,
            ):
                for h in range(NH):
                    for t in range(NT):
                        pc = pcp.tile([128, 512], f32, tag="pc",
                                      name=f"pc{t}_{h}")
                        psums = [
                            pep.tile([128, 512], f32, tag="ep",
                                     name=f"ep{t}_{h}_{e}")
                            for e in range(E)
                        ]

                        def corr_mms():
                            for k in range(KC):
                                nc.tensor.matmul(
                                    pc[:, :],
                                    xi_sb[:, k, :, ts(t, 128)],
                                    wc_sb[:, h, k, :, :],
                                    start=(k == 0),
                                    stop=(k == KC - 1),
                                    perf_mode=DR,
                                )

                        def expert_mms():
                            for kp in range(KP):
                                lhs = xi_sb[
                                    :, 2 * kp : 2 * kp + 2, 1, ts(t, 128)
                                ]
                                for e in range(E):
                                    nc.tensor.matmul(
                                        psums[e][:, :],
                                        lhs,
                                        w_sb[:, h, kp, :, e, :],
                                        start=(kp == 0),
                                        stop=(kp == KP - 1),
                                        perf_mode=DR,
                                    )

                        # correction first so its bank drains early; but the
                        # very first tile runs experts first (wt0 lands
                        # before wc0 in the DMA stream)
                        if h == 0 and t == 0:
                            expert_mms()
                            corr_mms()
                        else:
                            corr_mms()
                            expert_mms()
                        # combine: out = sum_e g_e*p_e + pc/512
                        g0 = gates_sb[:, t * E + 0 : t * E + 1]
                        g1 = gates_sb[:, t * E + 1 : t * E + 2]
                        g2 = gates_sb[:, t * E + 2 : t * E + 3]
                        g3 = gates_sb[:, t * E + 3 : t * E + 4]
                        ca = ap.tile([128, 512], f32, tag="ca")
                        cb = ap.tile([128, 512], f32, tag="cb")
                        acc = ap.tile([128, 512], bf16, tag="acc")
                        nc.scalar.activation(
                            ca[:, :],
                            pc[:, :],
                            mybir.ActivationFunctionType.Copy,
                            bias=0.0,
                            scale=1.0 / 512.0,
                        )
                        nc.vector.scalar_tensor_tensor(
                            ca[:, :], psums[0][:, :], g0, ca[:, :],
                            op0=mybir.AluOpType.mult,
                            op1=mybir.AluOpType.add,
                        )
                        nc.scalar.activation(
                            cb[:, :],
                            psums[1][:, :],
                            mybir.ActivationFunctionType.Copy,
                            bias=0.0,
                            scale=g1,
                        )
                        nc.vector.scalar_tensor_tensor(
                            cb[:, :], psums[2][:, :], g2, cb[:, :],
                            op0=mybir.AluOpType.mult,
                            op1=mybir.AluOpType.add,
                        )
                        nc.vector.scalar_tensor_tensor(
                            ca[:, :], psums[3][:, :], g3, ca[:, :],
                            op0=mybir.AluOpType.mult,
                            op1=mybir.AluOpType.add,
                        )
                        nc.vector.tensor_add(acc[:, :], ca[:, :], cb[:, :])
                        nc.sync.dma_start(
                            out=out_d[ts(t, 128), ds(512 * h, 512)],
                            in_=acc[:, :],
                        )
    nc.compile()
    return nc


def _prep_inputs(x, W_experts, gate_w, gate_b):
    x_flat = np.asarray(x, dtype=np.float32).reshape(N, D)
    Wf = np.asarray(W_experts, dtype=np.float32)        # [E, O, D]
    gwf = np.asarray(gate_w, dtype=np.float32)          # [E, D]
    gbf = np.asarray(gate_b, dtype=np.float32)          # [E]

    # host routing: top-1 expert per token (layout decision only)
    logits = x_flat @ gwf.T + gbf
    top = np.argmax(logits, -1)
    gh = np.exp(logits - logits.max(-1, keepdims=True))
    gh /= gh.sum(-1, keepdims=True)
    m_top = float(gh[np.arange(N), top].mean())
    m_off = (1.0 - m_top) / 3.0

    core_tokens = [None] * N_CORES
    spill = []
    for e in range(E):
        toks = np.where(top == e)[0]
        take = toks[: 2 * T]
        spill.append(toks[2 * T :])
        core_tokens[2 * e] = take[:T]
        core_tokens[2 * e + 1] = take[T : 2 * T]
    spill = np.concatenate(spill) if spill else np.empty(0, np.int64)
    for c in range(N_CORES):
        need = T - len(core_tokens[c])
        if need > 0:
            core_tokens[c] = np.concatenate([core_tokens[c], spill[:need]])
            spill = spill[need:]
    perm = np.concatenate(core_tokens)

    # weights (shared across cores)
    W8 = (Wf * S_W).astype(FP8)                         # [E, O, D]
    W8f = np.asarray(W8, dtype=np.float32) / S_W
    dW = Wf - W8f
    # wt[h, kp, p, j, e, o5] = W8[e, 512h+o5, (2kp+j)*128+p]
    wt = np.ascontiguousarray(
        W8.reshape(E, NH, 512, KP, 2, 128).transpose(1, 3, 5, 4, 0, 2)
    )
    gw8 = (gwf.T * S_GW).astype(FP8)                    # [D, E]
    dgw = gwf.T - np.asarray(gw8, dtype=np.float32) / S_GW
    dgw8 = (dgw * S_DGW).astype(FP8)
    Gcat = np.zeros((D, 2, GP), dtype=FP8)
    Gcat[:, 0, 0:E] = np.asarray(gw8)
    Gcat[:, 1, 0:E] = np.asarray(dgw8)
    gs = np.ascontiguousarray(
        Gcat.reshape(KP, 2, 128, 2, GP).transpose(0, 2, 1, 3, 4)
    )
    gb = gbf.reshape(E, 1)
    cst = np.full((E, 1), 1.0 / 64.0, dtype=np.float32)
    ident = np.eye(E, dtype=np.float32).astype(BF16)

    # per-designated-expert correction matrices
    wcs = []
    for eh in range(E):
        m = np.full(E, m_off, dtype=np.float32)
        m[eh] = m_top
        A = np.einsum("e,eod->do", m, Wf)               # [D, O]
        Bm = np.einsum("e,eod->do", m, dW)
        A8 = (A * S_A).astype(FP8)
        B8 = (Bm * S_B).astype(FP8)
        # wc[h, k, p, j, o5]: j0 = A8 chunk, j1 = B8 chunk
        A8r = np.asarray(A8).reshape(KC, 128, NH, 512).transpose(2, 0, 1, 3)
        B8r = np.asarray(B8).reshape(KC, 128, NH, 512).transpose(2, 0, 1, 3)
        wcs.append(np.ascontiguousarray(np.stack([A8r, B8r], axis=3)))

    in_maps = []
    for c in range(N_CORES):
        idx = core_tokens[c]
        xc = x_flat[idx]                                # [T, D]
        x8 = xc.astype(FP8)
        dx = xc - np.asarray(x8, dtype=np.float32)
        dx8 = (dx * S_DX).astype(FP8)
        x8r = np.asarray(x8).T.reshape(KC, 128, T)
        dx8r = np.asarray(dx8).T.reshape(KC, 128, T)
        xi = np.ascontiguousarray(np.stack([dx8r, x8r], axis=2))
        in_maps.append(
            {
                "xi": xi,
                "wt": wt,
                "wc": wcs[c // 2],
                "gs": gs,
                "gb": gb,
                "cst": cst,
                "ident": ident,
            }
        )
    return in_maps, perm


def _run(inputs, trace=False):
    from concourse.bass_utils import run_bass_kernel_spmd

    if "nc" not in _CACHE:
        _CACHE["nc"] = _build_graph()
    nc = _CACHE["nc"]
    in_maps, perm = _prep_inputs(**inputs)
    res = run_bass_kernel_spmd(
        nc, in_maps, core_ids=list(range(N_CORES)), trace=trace
    )
    out = np.empty((N, O), dtype=np.float32)
    for c in range(N_CORES):
        shard = np.asarray(res.results[c]["out"], dtype=np.float32)
        out[perm[c * T : (c + 1) * T]] = shard
    return out.reshape(B, S, O), res


def kernel(x, W_experts, gate_w, gate_b):
    out, _ = _run(
        {"x": x, "W_experts": W_experts, "gate_w": gate_w, "gate_b": gate_b}
    )
    return out
